# revision 1
# baseline (speedup 1.0000x reference)
"""Bag-of-words per-row histogram kernel for Trainium2 (8 NeuronCores).

Problem: input_ids [2048, 512] int64, vocab 30522, pad token 0.
Output: [2048, 30522] f32 where out[b, v] = count of v among tokens of row b
strictly before the first pad token.

Strategy (data parallel over batch, 256 rows per core):
  For each row, the histogram over 30522 bins is computed as a rank-1-sum
  factorization on the TensorEngine: write id = hi*240 + lo (hi<128, lo<240).
  Per 128-token chunk build one-hot matrices A[t, hi] and B[t, lo] (DVE
  tensor_scalar is_equal against iota tables, bf16 => 4x perf mode), then
  psum[hi, lo] += A^T @ B accumulates the row histogram (4 chunks of K=128).
  Validity (tokens before first pad) is folded into A by setting hi=-1 for
  invalid tokens on the host. ScalarE copies PSUM->SBUF as fp16 (counts <=
  512 are fp16-exact), and SWDGE DMA casts fp16->f32 into the padded DRAM
  output [256, 128*240=30720]; the host slices to 30522 and stacks cores.
"""

import sys

if "/opt/trn_rl_repo" not in sys.path:
    sys.path.insert(0, "/opt/trn_rl_repo")

import numpy as np

import concourse.bass as bass  # noqa: F401  (AP helpers)
import concourse.bacc as bacc
import concourse.mybir as mybir
import concourse.tile as tile
from concourse.bass_utils import run_bass_kernel_spmd

F32 = mybir.dt.float32
F16 = mybir.dt.float16
BF16 = mybir.dt.bfloat16

VOCAB = 30522
H, L = 128, 240           # id = hi*L + lo; padded bins H*L = 30720
B_FULL, S = 2048, 512
NCORES = 8
NROWS = B_FULL // NCORES  # 256 rows per core
NCHUNK = S // 128         # 4 K-chunks per row
PAIRS = NROWS // 2        # 2 rows per PSUM bank
GROUP = 4                 # pairs per output DMA (8 rows)

_last_results = None      # stash for test harness (exec_time_ns when traced)


def _build():
    nc = bacc.Bacc("TRN2", target_bir_lowering=False, debug=False,
                   num_devices=NCORES)
    hiT = nc.dram_tensor("hiT", [128, NCHUNK * NROWS], F32, kind="ExternalInput")
    loT = nc.dram_tensor("loT", [128, NCHUNK * NROWS], F32, kind="ExternalInput")
    out = nc.dram_tensor("out", [NROWS, H * L], F32, kind="ExternalOutput")

    with tile.TileContext(nc) as tc:
        with tc.tile_pool(name="const", bufs=1) as const_pool, \
             tc.tile_pool(name="idx", bufs=1) as idx_pool, \
             tc.tile_pool(name="oh", bufs=6) as oh_pool, \
             tc.tile_pool(name="stage", bufs=3) as stage_pool, \
             tc.tile_pool(name="psum", bufs=6, space="PSUM") as psum_pool:

            iota_h = const_pool.tile([128, H], BF16)
            nc.gpsimd.iota(iota_h[:, :], [[1, H]], channel_multiplier=0,
                           allow_small_or_imprecise_dtypes=True)
            iota_l = const_pool.tile([128, L], BF16)
            nc.gpsimd.iota(iota_l[:, :], [[1, L]], channel_multiplier=0,
                           allow_small_or_imprecise_dtypes=True)

            hiT_sb = idx_pool.tile([128, NCHUNK * NROWS], F32)
            nc.sync.dma_start(out=hiT_sb[:, :], in_=hiT.ap())
            loT_sb = idx_pool.tile([128, NCHUNK * NROWS], F32)
            nc.sync.dma_start(out=loT_sb[:, :], in_=loT.ap())

            for g in range(PAIRS // GROUP):
                st = stage_pool.tile([128, GROUP * 2 * L], F16)
                for k in range(GROUP):
                    pair = g * GROUP + k
                    ps = psum_pool.tile([128, 512], F32)
                    for sub in range(2):
                        r = pair * 2 + sub
                        for c in range(NCHUNK):
                            j = c * NROWS + r
                            a_t = oh_pool.tile([128, H], BF16, tag="a")
                            nc.vector.tensor_scalar(
                                a_t[:, :], iota_h[:, :],
                                hiT_sb[:, j:j + 1], None,
                                mybir.AluOpType.is_equal)
                            b_t = oh_pool.tile([128, L], BF16, tag="b")
                            nc.vector.tensor_scalar(
                                b_t[:, :], iota_l[:, :],
                                loT_sb[:, j:j + 1], None,
                                mybir.AluOpType.is_equal)
                            nc.tensor.matmul(
                                ps[:, sub * L:(sub + 1) * L],
                                a_t[:, :], b_t[:, :],
                                start=(c == 0), stop=(c == NCHUNK - 1))
                    nc.scalar.activation(
                        st[:, k * 2 * L:(k + 1) * 2 * L], ps[:, 0:2 * L],
                        mybir.ActivationFunctionType.Copy)
                r0 = g * GROUP * 2
                dview = out.ap()[r0:r0 + GROUP * 2, :].rearrange(
                    "r (p f) -> p r f", p=H, f=L)
                nc.gpsimd.dma_start(
                    out=dview,
                    in_=st[:, :].rearrange("p (r f) -> p r f", f=L))
    nc.compile()
    return nc


_nc_cache = None


def _get_nc():
    global _nc_cache
    if _nc_cache is None:
        _nc_cache = _build()
    return _nc_cache


def kernel(input_ids) -> np.ndarray:
    global _last_results
    ids = np.asarray(input_ids)
    assert ids.shape == (B_FULL, S), ids.shape

    # Host-side input formatting: validity (tokens strictly before the first
    # pad), hi/lo digit split, and the token-major [128, NCHUNK*NROWS] layout
    # each core's DVE consumes directly.
    ids64 = ids.astype(np.int64)
    valid = np.cumprod(ids64 != 0, axis=1).astype(bool)   # [B, S]
    hi = ids64 // L
    lo = ids64 % L
    hi_m = np.where(valid, hi, -1).astype(np.float32)
    lo_f = lo.astype(np.float32)

    def to_core_layout(x):
        # [NROWS, S] -> [128, NCHUNK*NROWS]; [p, c*NROWS + r] = x[r, c*128 + p]
        t = x.T.reshape(NCHUNK, 128, NROWS).transpose(1, 0, 2)
        return np.ascontiguousarray(t.reshape(128, NCHUNK * NROWS))

    in_maps = []
    for cc in range(NCORES):
        sl = slice(cc * NROWS, (cc + 1) * NROWS)
        in_maps.append({"hiT": to_core_layout(hi_m[sl]),
                        "loT": to_core_layout(lo_f[sl])})

    nc = _get_nc()
    res = run_bass_kernel_spmd(nc, in_maps, core_ids=list(range(NCORES)))
    _last_results = res

    out = np.concatenate([res.results[cc]["out"] for cc in range(NCORES)], axis=0)
    return np.ascontiguousarray(out[:, :VOCAB], dtype=np.float32)
